# revision 1
# baseline (speedup 1.0000x reference)
"""DiffAttn (differential attention) Trainium2 Bass kernel.

Self-contained: kernel(**inputs) takes the FULL unsharded inputs as numpy
arrays and returns the FULL output [2, 4096, 128] float32.

Sharding: 8 cores = (batch in {0,1}) x (query-block of 1024 rows).
Each core projects Q, and K/V for only its OWN 1024-key block (the key
block is chosen equal to the query block, so a single per-core xq upload
feeds all three projections); the four cores sharing a batch then
AllGather the kT/V parts on-device, and each core runs the two
transposed-score softmaxes, the combined PV, and RMSNorm for its queries.

Layout strategy (the load-bearing decision): scores are computed
TRANSPOSED ([sk, sq], keys on partitions) so that exp(scores) can feed the
PV matmul directly as the streaming operand with V as stationary weights --
attention always contracts over sk, so the probability matrix must have sk
on partitions; producing it there directly avoids a PE transpose of the
full [sq, 4096] probability matrix per q-tile. Softmax row-sums are
recovered with a ones-stationary matmul, transposed back to per-partition
form (tiny [1,128] PE transposes) for the normalization, which happens
AFTER PV:   attn = U1/s1 - lam * U2/s2.
"""

import math
import os
import sys
from contextlib import ExitStack

import numpy as np

for _p in ("/root/.axon_site/_ro/trn_rl_repo", "/opt/trn_rl_repo"):
    if os.path.isdir(_p) and _p not in sys.path:
        sys.path.append(_p)

import ml_dtypes  # noqa: E402

import concourse.bass as bass  # noqa: E402
import concourse.mybir as mybir  # noqa: E402
import concourse.tile as tile  # noqa: E402
from concourse import bacc, bass_utils  # noqa: E402
from concourse.masks import make_identity  # noqa: E402

B, S, D, H = 2, 4096, 2048, 128
H2 = H // 2  # 64
P = 128
NCORES = 8
QSHARD = 1024  # q rows per core
DCH = D // P  # 16 d-chunks
NKCH = S // P  # 32 key chunks of 128
NGROUPS, GW = 2, 512  # q groups per core
NBLK, BLKW = 4, 1024  # key blocks for projections
NJ = GW // P  # 4 q sub-blocks of 128 per group

LAMBDA_INIT = 0.8 - 0.6 * math.exp(-0.3 * 12)
RMS_EPS = float(np.finfo(np.float32).eps)
SCALE = 1.0 / math.sqrt(H2)

F32 = mybir.dt.float32
BF16 = mybir.dt.bfloat16

AF = mybir.ActivationFunctionType
OP = mybir.AluOpType


def _emit(ctx: ExitStack, tc: "tile.TileContext", lam: float):
    nc = tc.nc

    # Each core projects K/V only for its own 1024-key block (== its q block,
    # so the single xq input feeds q, k and v projections), then the four
    # cores sharing a batch AllGather the kT/V parts.
    xq = nc.dram_tensor("xq", (D, QSHARD), BF16, kind="ExternalInput").ap()
    part_d = nc.dram_tensor("part_d", (2, P, BLKW), BF16).ap()
    full_d = nc.dram_tensor("full_d", (2 * NBLK, P, BLKW), BF16).ap()
    wqT = nc.dram_tensor("wqT", (D, H), BF16, kind="ExternalInput").ap()
    wkT = nc.dram_tensor("wkT", (D, H), BF16, kind="ExternalInput").ap()
    wvT = nc.dram_tensor("wvT", (D, H), BF16, kind="ExternalInput").ap()
    rmsw = nc.dram_tensor("rmsw", (H,), F32, kind="ExternalInput").ap()
    out_d = nc.dram_tensor("out", (QSHARD, H), F32, kind="ExternalOutput").ap()

    # ---- constant / persistent SBUF tiles ----
    consts = ctx.enter_context(tc.tile_pool(name="consts", bufs=1))
    persist = ctx.enter_context(tc.tile_pool(name="persist", bufs=1))

    ident = consts.tile([P, P], F32)
    make_identity(nc, ident)
    ones_bf = consts.tile([P, 1], BF16)
    nc.vector.memset(ones_bf, 1.0)
    rmsw_bc = consts.tile([P, H], F32)
    nc.sync.dma_start(
        out=rmsw_bc,
        in_=bass.AP(tensor=rmsw.tensor, offset=0, ap=[[0, P], [1, H]]),
    )
    # weight tiles: w_sb[p, c, h] = W?T[c*128 + p, h]; DMAs are issued in
    # first-use order further below (wk -> xq head -> wv -> xq tail -> wq) to
    # pull the collective dispatch as early as possible
    wq_sb = consts.tile([P, DCH, H], BF16)
    wk_sb = consts.tile([P, DCH, H], BF16)
    wv_sb = consts.tile([P, DCH, H], BF16)

    qT_sb = persist.tile([P, QSHARD], BF16)  # [h, sq]
    kT_sb = persist.tile([P, S], BF16)  # [h, sk]
    v_sb = persist.tile([P, NKCH, P], BF16)  # [sk%128, chunk, h]

    xpool = ctx.enter_context(tc.tile_pool(name="xstream", bufs=1))
    epool = ctx.enter_context(tc.tile_pool(name="epool", bufs=6))
    usb_pool = ctx.enter_context(tc.tile_pool(name="usb", bufs=2))
    small = ctx.enter_context(tc.tile_pool(name="small", bufs=8))
    outp = ctx.enter_context(tc.tile_pool(name="outp", bufs=4))
    attn_pool = ctx.enter_context(tc.tile_pool(name="attnp", bufs=2 * NGROUPS * NJ + 1))

    # ---- load xq once; project q, and this core's own-block kT/V ----
    xq_r = xq.rearrange("(c p) q -> p c q", p=P)
    xq_sb = xpool.tile([P, DCH, QSHARD], BF16, tag="xq", bufs=1)
    nc.sync.dma_start(out=wk_sb, in_=wkT.rearrange("(c p) h -> p c h", p=P))
    for c4 in range(4):
        nc.sync.dma_start(out=xq_sb[:, c4, :], in_=xq_r[:, c4, :])
    nc.sync.dma_start(out=wv_sb, in_=wvT.rearrange("(c p) h -> p c h", p=P))
    for qt in range(1, 4):
        nc.sync.dma_start(
            out=xq_sb[:, qt * 4 : (qt + 1) * 4, :], in_=xq_r[:, qt * 4 : (qt + 1) * 4, :]
        )
    nc.sync.dma_start(out=wq_sb, in_=wqT.rearrange("(c p) h -> p c h", p=P))

    kpart_sb = persist.tile([P, BLKW], BF16)
    vpart_sb = persist.tile([P, 8, P], BF16)
    # projection PSUM pools live only until the collective is dispatched
    with tc.tile_pool(name="pp_proj", space="PSUM", bufs=1) as pp_proj:
        for sl in range(2):
            kacc = pp_proj.tile([P, 512], F32, tag="kacc", bufs=2)
            for c in range(DCH):
                nc.tensor.matmul(
                    kacc,
                    wk_sb[:, c, :],
                    xq_sb[:, c, sl * 512 : (sl + 1) * 512],
                    start=(c == 0),
                    stop=(c == DCH - 1),
                )
            nc.scalar.copy(kpart_sb[:, sl * 512 : (sl + 1) * 512], kacc)
        # V natural layout, 4 subtiles at a time (one PSUM bank); accumulation
        # groups sharing a bank must not overlap, hence j-outer c-inner
        for hf in range(2):
            vacc = pp_proj.tile([P, 4, P], F32, tag="vacc", bufs=2)
            for j4 in range(4):
                j = hf * 4 + j4
                for c in range(DCH):
                    nc.tensor.matmul(
                        vacc[:, j4, :],
                        xq_sb[:, c, j * P : (j + 1) * P],
                        wv_sb[:, c, :],
                        start=(c == 0),
                        stop=(c == DCH - 1),
                    )
            nc.vector.tensor_copy(vpart_sb[:, hf * 4 : (hf + 1) * 4, :], vacc)

        # ---- AllGather kT/V parts across the 4 cores sharing a batch ----
        nc.sync.dma_start(out=part_d[0], in_=kpart_sb)
        nc.sync.dma_start(out=part_d[1], in_=vpart_sb.rearrange("p j h -> p (j h)"))
        nc.gpsimd.collective_compute(
            "AllGather",
            OP.bypass,
            replica_groups=[[0, 1, 2, 3], [4, 5, 6, 7]],
            ins=[part_d.opt()],
            outs=[full_d.opt()],
        )

        # qT projection overlaps the collective flight time
        for sl in range(2):
            qacc = pp_proj.tile([P, 512], F32, tag="kacc", bufs=2)
            for c in range(DCH):
                nc.tensor.matmul(
                    qacc,
                    wq_sb[:, c, :],
                    xq_sb[:, c, sl * 512 : (sl + 1) * 512],
                    start=(c == 0),
                    stop=(c == DCH - 1),
                )
            nc.scalar.copy(qT_sb[:, sl * 512 : (sl + 1) * 512], qacc)

    for r in range(NBLK):
        nc.sync.dma_start(out=kT_sb[:, r * BLKW : (r + 1) * BLKW], in_=full_d[2 * r])
        nc.sync.dma_start(
            out=v_sb[:, r * 8 : (r + 1) * 8, :],
            in_=full_d[2 * r + 1].rearrange("p (j h) -> p j h", j=8),
        )

    # attention PSUM pools (after proj pools close): s 4 + u 2 + sums 2 = 8
    pp_s = ctx.enter_context(tc.tile_pool(name="pp_s", space="PSUM", bufs=2))
    pp_u = ctx.enter_context(tc.tile_pool(name="pp_u", space="PSUM", bufs=1))
    pp_sum = ctx.enter_context(tc.tile_pool(name="pp_sum", space="PSUM", bufs=1))

    def attend_chunk(g, u_ps, sums_ps, kT_ap, v_ap, start, stop):
        """scores -> exp -> sums/U accumulation for one 128-key chunk."""
        q0 = g * GW
        s_ps = pp_s.tile([P, 2 * GW], F32, tag="s", name="s_ps")
        nc.tensor.matmul(s_ps[:, 0:GW], kT_ap[0:H2, :], qT_sb[0:H2, q0 : q0 + GW])
        nc.tensor.matmul(
            s_ps[:, GW : 2 * GW], kT_ap[H2:H, :], qT_sb[H2:H, q0 : q0 + GW]
        )
        e_sb = epool.tile([P, 2 * GW], BF16, tag="e", name="e_sb")
        nc.scalar.activation(e_sb, s_ps, AF.Exp, scale=SCALE)
        for hf in range(2):
            sl = slice(hf * GW, (hf + 1) * GW)
            nc.tensor.matmul(
                sums_ps[g * 32 : g * 32 + 1, sl],
                ones_bf,
                e_sb[:, sl],
                start=start,
                stop=stop,
            )
            nc.tensor.matmul(u_ps[:, sl], v_ap, e_sb[:, sl], start=start, stop=stop)

    u_sbs = []
    sums_acc = small.tile([1, NGROUPS, 2 * GW], F32, tag="sums_acc", bufs=1)
    sums_ps = pp_sum.tile([33, 2 * GW], F32, tag="sum")

    # ---- warm-up pass while the AllGather is in flight: run group 0's
    # attention against this core's OWN locally-projected key block. The
    # gathered pass below covers every block exactly once, so these results
    # are discarded -- the point is to keep PE/ACT busy (and the PE HAM
    # clock-gate warm) instead of idling through the collective.
    u_warm = pp_u.tile([P, 2 * GW], F32, tag="u", name="u_warm")
    for wi in range(16):
        g, c8 = divmod(wi, 8)
        q0 = g * GW
        s_ps = pp_s.tile([P, 2 * GW], F32, tag="s", name="s_warm")
        nc.tensor.matmul(
            s_ps[:, 0:GW],
            kpart_sb[0:H2, c8 * P : (c8 + 1) * P],
            qT_sb[0:H2, q0 : q0 + GW],
        )
        nc.tensor.matmul(
            s_ps[:, GW : 2 * GW],
            kpart_sb[H2:H, c8 * P : (c8 + 1) * P],
            qT_sb[H2:H, q0 : q0 + GW],
        )
        e_sb = epool.tile([P, 2 * GW], BF16, tag="e", name="e_warm")
        nc.scalar.activation(e_sb, s_ps, AF.Exp, scale=SCALE)
        for hf in range(2):
            sl = slice(hf * GW, (hf + 1) * GW)
            nc.tensor.matmul(
                u_warm[:, sl],
                vpart_sb[:, c8, :],
                e_sb[:, sl],
                start=(wi == 0),
                stop=(wi == 15),
            )

    # ---- the real attention: all four gathered blocks, per group ----
    for g in range(NGROUPS):
        u_ps = pp_u.tile([P, 2 * GW], F32, tag="u", name=f"u_ps{g}")
        for ch in range(NKCH):
            attend_chunk(
                g,
                u_ps,
                sums_ps,
                kT_sb[:, ch * P : (ch + 1) * P],
                v_sb[:, ch, :],
                start=(ch == 0),
                stop=(ch == NKCH - 1),
            )
        u_sb = usb_pool.tile([P, 2 * GW], F32, tag="usb")
        nc.vector.tensor_copy(u_sb, u_ps)
        nc.vector.tensor_copy(sums_acc[0:1, g, :], sums_ps[g * 32 : g * 32 + 1, :])
        u_sbs.append(u_sb)

    # ---- post phase: normalize + combine + RMS stats ----
    finals = []  # (attn_sb, rmsin_sb, row0)

    c_ = 1.0 - LAMBDA_INIT
    a_ = 1.0 / (H * c_ * c_)
    b_ = RMS_EPS / (c_ * c_)
    r_sb = small.tile([P, 2 * 2 * NJ], F32, tag="r", bufs=1)
    for g in range(NGROUPS):
        # sums -> per-partition layout via tiny PE transposes ("s"-tag psum
        # slots cycle quickly, letting group 0's post overlap group 1's tail)
        sumsT_ps = pp_s.tile([P, 2 * NJ], F32, tag="s")
        for hf in range(2):
            for j in range(NJ):
                nc.tensor.transpose(
                    sumsT_ps[:, hf * NJ + j : hf * NJ + j + 1],
                    sums_acc[0:1, g, hf * GW + j * P : hf * GW + (j + 1) * P],
                    ident[0:1, 0:1],
                )
        rg = r_sb[:, g * 2 * NJ : (g + 1) * 2 * NJ]
        nc.vector.reciprocal(rg, sumsT_ps)
        nc.vector.tensor_scalar_mul(
            r_sb[:, g * 2 * NJ + NJ : (g + 1) * 2 * NJ],
            r_sb[:, g * 2 * NJ + NJ : (g + 1) * 2 * NJ],
            lam,
        )

    for g in range(NGROUPS):
        post_ps = pp_u.tile([P, 2 * NJ, P], F32, tag="u")
        for j in range(NJ):
            nc.tensor.transpose(
                post_ps[:, j, :], u_sbs[g][:, j * P : (j + 1) * P], ident
            )
            nc.tensor.transpose(
                post_ps[:, NJ + j, :], u_sbs[g][:, GW + j * P : GW + (j + 1) * P], ident
            )
        for j in range(NJ):
            rcol = g * 2 * NJ
            t2 = small.tile([P, P], F32, tag="t2")
            nc.scalar.activation(
                t2,
                post_ps[:, NJ + j, :],
                AF.Copy,
                scale=r_sb[:, rcol + NJ + j : rcol + NJ + j + 1],
            )
            attn_sb = attn_pool.tile([P, P], F32, tag="attn")
            nc.vector.scalar_tensor_tensor(
                attn_sb,
                post_ps[:, j, :],
                r_sb[:, rcol + j : rcol + j + 1],
                t2,
                op0=OP.mult,
                op1=OP.subtract,
            )
            sq_scr = small.tile([P, P], F32, tag="sqscr")
            ssq = small.tile([P, 1], F32, tag="ssq")
            nc.scalar.activation(sq_scr, attn_sb, AF.Square, accum_out=ssq)
            rmsin = small.tile([P, 1], F32, tag="rmsin")
            nc.vector.tensor_scalar(rmsin, ssq, a_, b_, op0=OP.mult, op1=OP.add)
            finals.append((attn_sb, rmsin, g * GW + j * P))

    # ---- phase C: final normalization + store ----
    for attn_sb, rmsin, row0 in finals:
        root = small.tile([P, 1], F32, tag="root")
        nc.scalar.activation(root, rmsin, AF.Sqrt)
        rrms = small.tile([P, 1], F32, tag="rrms")
        nc.vector.reciprocal(rrms, root)
        o_sb = outp.tile([P, H], F32, tag="o")
        nc.vector.scalar_tensor_tensor(
            o_sb, attn_sb, rrms, rmsw_bc, op0=OP.mult, op1=OP.mult
        )
        nc.sync.dma_start(out=out_d[row0 : row0 + P, :], in_=o_sb)


def build(lam: float):
    from concourse._compat import axon_active

    # The axon/PJRT redirect path has no BassDebugger (no /dev/neuron* on the
    # client), so it needs debug=False; the native NrtSession path expects a
    # debug-enabled Bass (mirrors bass_test_utils.run_kernel).
    nc = bacc.Bacc(
        "TRN2",
        target_bir_lowering=False,
        debug=not axon_active(),
        num_devices=NCORES,
    )
    with tile.TileContext(nc) as tc:
        with ExitStack() as ctx:
            _emit(ctx, tc, lam)
    nc.compile()
    return nc


def make_in_maps(x, Wq, Wk, Wv, rms_weight):
    bf = ml_dtypes.bfloat16
    x = np.asarray(x, dtype=np.float32)
    xT = np.ascontiguousarray(x.transpose(0, 2, 1)).astype(bf)  # [B, D, S]
    wqT = np.ascontiguousarray(np.asarray(Wq, np.float32).T).astype(bf)
    wkT = np.ascontiguousarray(np.asarray(Wk, np.float32).T).astype(bf)
    wvT = np.ascontiguousarray(np.asarray(Wv, np.float32).T).astype(bf)
    rw = np.ascontiguousarray(np.asarray(rms_weight, np.float32))
    in_maps = []
    for core in range(NCORES):
        b, qb = divmod(core, NCORES // B)
        in_maps.append(
            {
                "xq": np.ascontiguousarray(xT[b][:, qb * QSHARD : (qb + 1) * QSHARD]),
                "wqT": wqT,
                "wkT": wkT,
                "wvT": wvT,
                "rmsw": rw,
            }
        )
    return in_maps


def kernel(x, Wq, Wk, Wv, lambda_q1, lambda_q2, lambda_k1, lambda_k2, rms_weight):
    lq1 = np.asarray(lambda_q1, np.float32)
    lq2 = np.asarray(lambda_q2, np.float32)
    lk1 = np.asarray(lambda_k1, np.float32)
    lk2 = np.asarray(lambda_k2, np.float32)
    lam = float(
        np.exp(np.dot(lq1, lk1)) - np.exp(np.dot(lq2, lk2)) + LAMBDA_INIT
    )
    nc = build(lam)
    in_maps = make_in_maps(x, Wq, Wk, Wv, rms_weight)
    res = bass_utils.run_bass_kernel_spmd(nc, in_maps, core_ids=list(range(NCORES)))
    out = np.empty((B, S, H), np.float32)
    for core in range(NCORES):
        b, qb = divmod(core, NCORES // B)
        out[b, qb * QSHARD : (qb + 1) * QSHARD] = res.results[core]["out"]
    return out



# revision 15
# speedup vs baseline: 1.6135x; 1.6135x over previous
"""DiffAttn (differential attention) Trainium2 Bass kernel — v2.

Self-contained: kernel(**inputs) takes the FULL unsharded inputs as numpy
arrays and returns the FULL output [2, 4096, 128] float32.

Sharding: 8 cores = (batch in {0,1}) x (query-block of 1024 rows). Each core
is FULLY independent (no collectives, no cross-core traffic): it loads the
whole batch's activations (x^T, host-permuted to [p][c][q] with its own query
block's columns first), projects K and V for all 4096 keys locally, projects
Q for its own 1024 queries, and runs both softmaxes + the combined PV +
RMSNorm for its queries.  Replicating the K/V projections (4x) costs ~41us of
PE time but removes the AllGather entirely, which the timeline model prices
at ~67us mostly unoverlappable — and it makes every core's program trivially
SPMD-identical.

Layout strategy: scores are computed TRANSPOSED ([sk, q], keys on partitions)
so exp(scores) feeds the PV matmul as the STATIONARY operand (e[sk, q128]
slices) against a MOVING V_aug = [V | ones] (129 cols).  The ones column
makes each PV accumulation also produce the softmax denominator in column
128 of U — no separate row-sum matmuls (saves 64x1024 PE rows) and no PE
transposes in the post phase (U is already [q, h]).

Key order is irrelevant to attention, so K/V key chunks can be processed in
any order — here in the host-permuted order of xb.

Post phase identity used:  attn = U1/s1 - lam*U2/s2; RMS-norm of attn equals
RMS-norm of w := s1*attn = U1 - lam*(s1/s2)*U2 (the 1/s1 scale cancels in
x*rsqrt(mean(x^2)); torch's eps=1.19e-7 is ~1e-5 of mean(attn^2) here, far
below the 2e-2 gate).  out = w * rsqrt(sum_h w^2) * [rmsw*(1-li)*sqrt(H)].
All of the post runs on DVE/Pool except one batched Rsqrt at the very end
(one ACT table switch).

PSUM budget (8 banks of [128 x 2KB]): attention uses s(3 bufs of [128,512])
+ 4 U slot banks + 1 "work" bank for the next block's K/V projection
accumulators; block 0's K/Q projections run c-inner in 4 dedicated banks
before attention starts so the PE ramps while x streams in.
"""

import math
import os
import sys
from contextlib import ExitStack

import numpy as np

for _p in ("/root/.axon_site/_ro/trn_rl_repo", "/opt/trn_rl_repo"):
    if os.path.isdir(_p) and _p not in sys.path:
        sys.path.append(_p)

import ml_dtypes  # noqa: E402

import concourse.bass as bass  # noqa: E402
import concourse.mybir as mybir  # noqa: E402
import concourse.tile as tile  # noqa: E402
from concourse import bacc, bass_utils  # noqa: E402

B, S, D, H = 2, 4096, 2048, 128
H2 = H // 2  # 64
P = 128
NCORES = 8
QSHARD = 1024  # q rows per core
DCH = D // P  # 16 d-chunks
NKCH = S // P  # 32 key chunks of 128
NBLK, BLKW = 4, 1024  # key blocks (1024 keys each)
NPASS, PW = 4, 256  # query passes of 256 columns (U psum = 4 banks)

LAMBDA_INIT = 0.8 - 0.6 * math.exp(-0.3 * 12)
SCALE = 1.0 / math.sqrt(H2)

F32 = mybir.dt.float32
BF16 = mybir.dt.bfloat16

AF = mybir.ActivationFunctionType
OP = mybir.AluOpType


def _emit(ctx: ExitStack, tc: "tile.TileContext", lam: float):
    nc = tc.nc

    xb_d = nc.dram_tensor("xb", (P, DCH, S), BF16, kind="ExternalInput").ap()
    wqT = nc.dram_tensor("wqT", (P, DCH, H), BF16, kind="ExternalInput").ap()
    wkT = nc.dram_tensor("wkT", (P, DCH, H), BF16, kind="ExternalInput").ap()
    wvT = nc.dram_tensor("wvT", (P, DCH, H), BF16, kind="ExternalInput").ap()
    rmsw = nc.dram_tensor("rmsw", (H,), F32, kind="ExternalInput").ap()
    out_d = nc.dram_tensor("out", (QSHARD, H), F32, kind="ExternalOutput").ap()

    # ---- persistent SBUF ----
    consts = ctx.enter_context(tc.tile_pool(name="consts", bufs=1))
    persist = ctx.enter_context(tc.tile_pool(name="persist", bufs=1))

    wk_sb = consts.tile([P, DCH, H], BF16)
    wv_sb = consts.tile([P, DCH, H], BF16)
    wq_sb = consts.tile([P, DCH, H], BF16)
    rmsw_bc = consts.tile([P, H], F32)

    kT_sb = persist.tile([P, S], BF16)  # [h, key]
    qT_sb = persist.tile([P, QSHARD], BF16)  # [h, q]
    # second-half heads relocated to base partition 0 (PE tile-position rule:
    # matmuls sharing a PSUM bank must share a row base)
    kT2_sb = persist.tile([64, S], BF16)
    qT2_sb = persist.tile([64, QSHARD], BF16)
    v_sb = persist.tile([P, NKCH, H + 1], BF16)  # [sk%128, chunk, h|1]
    u_acc = persist.tile([P, NPASS, 2, 2, H + 1], F32)  # [q, pass, hf, j2, h|s]
    w_all = persist.tile([P, 2 * NPASS, H], F32)  # [q, subtile, h]
    rsq_all = persist.tile([P, 2 * NPASS], F32)
    rr_all = persist.tile([P, 2 * NPASS], F32)

    xpool = ctx.enter_context(tc.tile_pool(name="xstream", bufs=1))
    epool = ctx.enter_context(tc.tile_pool(name="epool", bufs=6))
    small = ctx.enter_context(tc.tile_pool(name="small", bufs=4))
    outp = ctx.enter_context(tc.tile_pool(name="outp", bufs=4))

    # ---- const DMAs; ones column of V_aug ----
    nc.sync.dma_start(out=wk_sb, in_=wkT)
    nc.vector.memset(v_sb[:, :, H : H + 1], 1.0)
    nc.sync.dma_start(
        out=rmsw_bc,
        in_=bass.AP(tensor=rmsw.tensor, offset=0, ap=[[0, P], [1, H]]),
    )

    # ---- x block streams: block 0 per-chunk (projection pipelines with the
    # DMA), blocks 1-3 in 4-chunk slabs ----
    x_tiles = []
    for b in range(NBLK):
        x_tiles.append(
            xpool.tile([P, DCH, BLKW], BF16, tag=f"x{b % 3}", bufs=1, name=f"xt{b}")
        )
    for c in range(DCH):
        nc.sync.dma_start(
            out=x_tiles[0][:, c, :], in_=xb_d[:, c, 0:BLKW]
        )
        if c == 0:
            nc.sync.dma_start(out=wq_sb, in_=wqT)
        if c == 2:
            nc.sync.dma_start(out=wv_sb, in_=wvT)

    psum = ctx.enter_context(tc.tile_pool(name="psum", space="PSUM", bufs=1))

    def proj_copy_k(acc, b, g):
        sl = slice(b * BLKW + g * 512, b * BLKW + (g + 1) * 512)
        nc.vector.tensor_copy(kT_sb[:, sl], acc)
        nc.sync.dma_start(out=kT2_sb[:, sl], in_=kT_sb[64:128, sl])

    # ---- block 0: K and Q projections c-inner (4 psum banks), pipelined with
    # the per-chunk x DMAs so the PE starts early and ramps ----
    # K/Q accumulators for block 0 borrow the four U slot banks (attention
    # hasn't started; the tile pool serializes reuse via dependencies)
    kacc0 = [
        psum.tile([P, 512], F32, tag=f"u{g}", bufs=1, name=f"kacc{g}")
        for g in range(2)
    ]
    qacc0 = [
        psum.tile([P, 512], F32, tag=f"u{2 + g}", bufs=1, name=f"qacc{g}")
        for g in range(2)
    ]
    for c in range(DCH):
        st, sp = c == 0, c == DCH - 1
        for g in range(2):
            sl = slice(g * 512, (g + 1) * 512)
            nc.tensor.matmul(
                kacc0[g], wk_sb[:, c, :], x_tiles[0][:, c, sl], start=st, stop=sp
            )
            nc.tensor.matmul(
                qacc0[g], wq_sb[:, c, :], x_tiles[0][:, c, sl], start=st, stop=sp
            )
    for g in range(2):
        proj_copy_k(kacc0[g], 0, g)
        nc.vector.tensor_copy(qT_sb[:, g * 512 : (g + 1) * 512], qacc0[g])
    nc.sync.dma_start(out=qT2_sb, in_=qT_sb[64:128, :])

    # remaining x DMAs (xpool tags cycle: x0 reused by b3 after b0 released)
    for b in range(1, NBLK):
        for c4 in range(4):
            nc.sync.dma_start(
                out=x_tiles[b][:, c4 * 4 : (c4 + 1) * 4, :],
                in_=xb_d[:, c4 * 4 : (c4 + 1) * 4, b * BLKW : (b + 1) * BLKW],
            )

    def vproj_block(b):
        """V projection for block b, j-outer (sequential psum groups in ONE
        work bank), x(b) fully resident."""
        for g in range(2):
            vacc = psum.tile([P, 4, P], F32, tag="work", bufs=1)
            for j in range(4):
                for c in range(DCH):
                    nc.tensor.matmul(
                        vacc[:, j, :],
                        x_tiles[b][:, c, g * 512 + j * P : g * 512 + (j + 1) * P],
                        wv_sb[:, c, :],
                        start=(c == 0),
                        stop=(c == DCH - 1),
                    )
            nc.vector.tensor_copy(
                v_sb[:, b * 8 + g * 4 : b * 8 + (g + 1) * 4, 0:H], vacc
            )

    def kproj_block(b):
        """K projection for block b in the work bank (sequential groups)."""
        for g in range(2):
            acc = psum.tile([P, 512], F32, tag="work", bufs=1)
            for c in range(DCH):
                nc.tensor.matmul(
                    acc,
                    wk_sb[:, c, :],
                    x_tiles[b][:, c, g * 512 : (g + 1) * 512],
                    start=(c == 0),
                    stop=(c == DCH - 1),
                )
            proj_copy_k(acc, b, g)

    vproj_block(0)

    sub = os.environ.get("KSUB", "full")

    def attend_block_pass(b, p):
        """Pass p (256 q cols), all 8 key chunks of block b -> U psum, then
        drain-add into u_acc."""
        q0 = p * PW
        nslots = 4 if sub in ("full", "pv4") else (1 if sub == "pv1" else 0)
        u_ps = [
            psum.tile([P, 512], F32, tag=f"u{s}", bufs=1, name=f"u{b}_{p}_{s}")
            for s in range(nslots)
        ]
        for cl in range(8):
            ch = b * 8 + cl
            s_ps = psum.tile([P, 512], F32, tag="s", bufs=3)
            nc.tensor.matmul(
                s_ps[:, 0:PW],
                kT_sb[0:H2, ch * P : (ch + 1) * P],
                qT_sb[0:H2, q0 : q0 + PW],
            )
            nc.tensor.matmul(
                s_ps[:, PW : 2 * PW],
                kT2_sb[:, ch * P : (ch + 1) * P],
                qT2_sb[:, q0 : q0 + PW],
            )
            e_sb = epool.tile([P, 512], BF16, tag="e")
            nc.scalar.activation(e_sb, s_ps, AF.Exp, scale=SCALE)
            if sub in ("sonly", "noattn"):
                continue
            for hf in range(2):
                for j2 in range(2):
                    slot = hf * 2 + j2
                    if slot >= nslots:
                        continue
                    nc.tensor.matmul(
                        u_ps[slot][:, 0 : H + 1],
                        e_sb[:, hf * PW + j2 * P : hf * PW + (j2 + 1) * P],
                        v_sb[:, ch, :],
                        start=(cl == 0),
                        stop=(cl == 7),
                    )
        for slot in range(nslots):
            hf, j2 = divmod(slot, 2)
            dst = u_acc[:, p, hf, j2, :]
            src = u_ps[slot][:, 0 : H + 1]
            if b == 0:
                nc.vector.tensor_copy(dst, src)
            else:
                nc.vector.tensor_tensor(dst, src, dst, op=OP.add)

    def post_pass(p):
        """Normalize/combine for pass p's two q-subtiles (DVE + Pool only)."""
        for j2 in range(2):
            t = p * 2 + j2
            s1 = u_acc[:, p, 0, j2, H : H + 1]
            s2 = u_acc[:, p, 1, j2, H : H + 1]
            r2 = small.tile([P, 1], F32, tag="r2")
            nc.vector.reciprocal(r2, s2)
            al = small.tile([P, 1], F32, tag="al")
            nc.vector.scalar_tensor_tensor(
                al, s1, -lam, r2, op0=OP.mult, op1=OP.mult
            )
            nc.vector.scalar_tensor_tensor(
                w_all[:, t, :],
                u_acc[:, p, 1, j2, 0:H],
                al,
                u_acc[:, p, 0, j2, 0:H],
                op0=OP.mult,
                op1=OP.add,
            )
            sq = small.tile([P, H], F32, tag="sq")
            nc.gpsimd.tensor_tensor(sq, w_all[:, t, :], w_all[:, t, :], op=OP.mult)
            nc.vector.tensor_reduce(
                rsq_all[:, t : t + 1], sq, axis=mybir.AxisListType.X, op=OP.add
            )

    stage = int(os.environ.get("KSTAGE", "3"))
    if stage <= 1:
        # debug: projections only
        for b in range(1, NBLK):
            kproj_block(b)
            vproj_block(b)
        o_dbg = outp.tile([P, H], F32, tag="o")
        nc.vector.tensor_copy(o_dbg, kT_sb[:, 0:H])
        for t in range(2 * NPASS):
            nc.sync.dma_start(out=out_d[t * P : (t + 1) * P, :], in_=o_dbg)
        return

    # ---- main schedule: per block: (next block's K proj) + 4 passes +
    # (next block's V proj) interleaved between passes ----
    interleave = os.environ.get("KILV", "1") == "1"
    nblk_run = int(os.environ.get("KNBLK", str(NBLK)))
    npass_run = int(os.environ.get("KNPASS", str(NPASS)))
    if not interleave:
        for b in range(1, NBLK):
            kproj_block(b)
            vproj_block(b)
    for b in range(nblk_run):
        for p in range(npass_run):
            if interleave and b < NBLK - 1:
                if p == 1:
                    kproj_block(b + 1)
                elif p == 2:
                    vproj_block(b + 1)
            attend_block_pass(b, p)
            if b == NBLK - 1 and stage >= 3:
                post_pass(p)
    if stage == 2:
        # debug: attention only; dump kT (always written)
        for t in range(2 * NPASS):
            o_dbg = outp.tile([P, H], F32, tag="o")
            nc.vector.tensor_copy(o_dbg, kT_sb[:, 0:H])
            nc.sync.dma_start(out=out_d[t * P : (t + 1) * P, :], in_=o_dbg)
        return

    # ---- batched sqrt (one ACT table switch) + recip, final scale, store ----
    root_all = persist.tile([P, 2 * NPASS], F32)
    nc.scalar.activation(root_all, rsq_all, AF.Sqrt)
    nc.vector.reciprocal(rr_all, root_all)
    for t in range(2 * NPASS):
        o_sb = outp.tile([P, H], F32, tag="o")
        nc.vector.scalar_tensor_tensor(
            o_sb, w_all[:, t, :], rr_all[:, t : t + 1], rmsw_bc,
            op0=OP.mult, op1=OP.mult,
        )
        nc.sync.dma_start(out=out_d[t * P : (t + 1) * P, :], in_=o_sb)


def build(lam: float):
    from concourse._compat import axon_active

    nc = bacc.Bacc(
        "TRN2",
        target_bir_lowering=False,
        debug=not axon_active(),
        num_devices=NCORES,
    )
    with tile.TileContext(nc) as tc:
        with ExitStack() as ctx:
            _emit(ctx, tc, lam)
    nc.compile()
    return nc


def make_in_maps(x, Wq, Wk, Wv, rms_weight):
    bf = ml_dtypes.bfloat16
    x = np.asarray(x, dtype=np.float32)
    # [B, D, S] -> [B][p][c][q] with c the d-chunk index
    xT = np.ascontiguousarray(x.transpose(0, 2, 1))  # [B, D, S]
    xP = xT.reshape(B, DCH, P, S).transpose(0, 2, 1, 3)  # [B, p, c, S]

    def wperm(W):
        # w[p, c, h] = W[h, c*128+p]
        WT = np.asarray(W, np.float32).T  # [D, H]
        return np.ascontiguousarray(
            WT.reshape(DCH, P, H).transpose(1, 0, 2)
        ).astype(bf)

    wq_p, wk_p, wv_p = wperm(Wq), wperm(Wk), wperm(Wv)
    c_ = 1.0 - LAMBDA_INIT
    rw = np.ascontiguousarray(
        np.asarray(rms_weight, np.float32) * c_ * math.sqrt(H)
    ).astype(np.float32)

    in_maps = []
    for core in range(NCORES):
        b, qb = divmod(core, NCORES // B)
        # own query block's columns first; key order is attention-irrelevant
        cols = np.r_[qb * QSHARD : (qb + 1) * QSHARD,
                     0 : qb * QSHARD, (qb + 1) * QSHARD : S]
        xb = np.ascontiguousarray(xP[b][:, :, cols]).astype(bf)
        in_maps.append(
            {"xb": xb, "wqT": wq_p, "wkT": wk_p, "wvT": wv_p, "rmsw": rw}
        )
    return in_maps


def kernel(x, Wq, Wk, Wv, lambda_q1, lambda_q2, lambda_k1, lambda_k2, rms_weight):
    lq1 = np.asarray(lambda_q1, np.float32)
    lq2 = np.asarray(lambda_q2, np.float32)
    lk1 = np.asarray(lambda_k1, np.float32)
    lk2 = np.asarray(lambda_k2, np.float32)
    lam = float(
        np.exp(np.dot(lq1, lk1)) - np.exp(np.dot(lq2, lk2)) + LAMBDA_INIT
    )
    nc = build(lam)
    in_maps = make_in_maps(x, Wq, Wk, Wv, rms_weight)
    res = bass_utils.run_bass_kernel_spmd(nc, in_maps, core_ids=list(range(NCORES)))
    out = np.empty((B, S, H), np.float32)
    for core in range(NCORES):
        b, qb = divmod(core, NCORES // B)
        out[b, qb * QSHARD : (qb + 1) * QSHARD] = res.results[core]["out"]
    return out


# revision 32
# speedup vs baseline: 1.7951x; 1.1126x over previous
"""DiffAttn (differential attention) Trainium2 Bass kernel — v3.

Self-contained: kernel(**inputs) takes the FULL unsharded inputs as numpy
arrays and returns the FULL output [2, 4096, 128] float32.

Sharding: 8 cores = (batch in {0,1}) x (query-block of 1024 rows). Each core
is FULLY independent (no collectives): it streams the whole batch's
activations (host-permuted to [p][c][q] with its own query block's columns
first), projects K and V for all 4096 keys locally, Q for its own 1024
queries, and runs both softmaxes + combined PV + RMSNorm for its queries.
Replicating the K/V projections beats the AllGather under the timeline model
(the collective is priced at ~67us, mostly unoverlappable) and keeps every
core's program trivially SPMD-identical.

fp8 projections (DoubleRow, 4x PE throughput): x and the weights are shipped
as e4m3 (weights pre-scaled by 64 so they sit in fp8's normal range; the 64^2
on the scores is folded into the softmax exp scale, exactly).  Q/K tolerate
plain fp8 (softmax output error ~0.1%).  V needs more precision, so V is
computed as  x8@wv8 + (x8@ws8 + r8@wv8)/32  where r8 = fp8(32*(x - x8)) and
ws8 = fp8(32*(wv' - wv8)) — a first-order residual expansion; the dropped
r*s cross term is ~0.1%.  The global 64x on V cancels in RMSNorm.

Attention layout: scores are computed TRANSPOSED ([sk, q], keys on
partitions) so exp(scores) feeds PV as the STATIONARY operand against a
MOVING V_aug = [V | ones] (129 cols); the ones column makes each PV
accumulation also produce the softmax denominator in column 128 of U — no
separate row-sum matmuls and no transposes in the post phase.  Key order is
attention-irrelevant, so K/V chunks process in host-permuted order.
Second-half heads live in base-partition-0 copies (kT2/qT2, via SBUF
partition-move DMAs): matmuls that share a PSUM bank must share a PE row
base (hardware tile-position rule).

Post: attn = U1/s1 - lam*U2/s2; RMSNorm(attn) == RMSNorm(s1*attn), so
w := U1 - lam*(s1/s2)*U2 needs no 1/s1 division (torch eps is ~1e-5 of
mean(attn^2) here).  out = w * rsqrt(sum_h w^2) * [rmsw*(1-li)*sqrt(H)].
rsqrt runs on DVE (bit-trick + 2 Newton steps, ~5e-6 rel) so the ACT exp
stream never switches activation tables; the post runs on DVE/Pool per pass,
fully overlapped except the last pass's short chain.

PSUM (8 banks): s(2 bufs, 1 bank each) + 4 U slot banks + work(2 bufs) for
the next block's K/V accumulators.  Block 0's K/Q run c-inner in the four
U banks (idle until attention starts) so the PE ramps while x streams in.
"""

import math
import os
import sys
from contextlib import ExitStack

import numpy as np

for _p in ("/root/.axon_site/_ro/trn_rl_repo", "/opt/trn_rl_repo"):
    if os.path.isdir(_p) and _p not in sys.path:
        sys.path.append(_p)

import ml_dtypes  # noqa: E402

import concourse.bass as bass  # noqa: E402
import concourse.mybir as mybir  # noqa: E402
import concourse.tile as tile  # noqa: E402
from concourse import bacc, bass_utils  # noqa: E402

B, S, D, H = 2, 4096, 2048, 128
H2 = H // 2  # 64
P = 128
NCORES = 8
QSHARD = 1024  # q rows per core
DCH = D // P  # 16 d-chunks
NKCH = S // P  # 32 key chunks of 128
NBLK, BLKW = 4, 1024  # key blocks (1024 keys each)
NPASS, PW = 4, 256  # query passes of 256 columns (U psum = 4 banks)
WSCALE = 64.0  # host-side weight prescale (exact power of 2)

LAMBDA_INIT = 0.8 - 0.6 * math.exp(-0.3 * 12)
SCALE = 1.0 / math.sqrt(H2)
SCALE_EXP = SCALE / (WSCALE * WSCALE)  # exp input is (64q).(64k)

F32 = mybir.dt.float32
BF16 = mybir.dt.bfloat16
F8 = mybir.dt.float8e4
I32 = mybir.dt.int32

AF = mybir.ActivationFunctionType
OP = mybir.AluOpType
DR = mybir.MatmulPerfMode.DoubleRow


def _emit(ctx: ExitStack, tc: "tile.TileContext", lam: float):  # noqa: C901
    nc = tc.nc

    x8_d = nc.dram_tensor("x8", (P, DCH, S), F8, kind="ExternalInput").ap()
    r8_d = nc.dram_tensor("r8", (P, DCH, S), F8, kind="ExternalInput").ap()
    wpk_d = nc.dram_tensor("wpk", (P, 6, DCH, H), F8, kind="ExternalInput").ap()
    rmsw = nc.dram_tensor("rmsw", (H,), F32, kind="ExternalInput").ap()
    out_d = nc.dram_tensor("out", (QSHARD, H), F32, kind="ExternalOutput").ap()

    # ---- persistent SBUF ----
    consts = ctx.enter_context(tc.tile_pool(name="consts", bufs=1))
    persist = ctx.enter_context(tc.tile_pool(name="persist", bufs=1))

    wpk_sb = consts.tile([P, 6, DCH, H], F8)
    wk_sb, wq_sb, wks_sb, wqs_sb, wv_sb, ws_sb = (
        wpk_sb[:, i] for i in range(6)
    )
    rmsw_bc = consts.tile([P, H], F32)

    kT_sb = persist.tile([P, S], BF16)  # [h, key]
    qT_sb = persist.tile([P, QSHARD], BF16)  # [h, q]
    # second-half heads relocated to base partition 0 (PE tile-position rule)
    kT2_sb = persist.tile([64, S], BF16)
    qT2_sb = persist.tile([64, QSHARD], BF16)
    v_sb = persist.tile([P, NKCH, H + 1], BF16)  # [sk%128, chunk, h|1]
    u_acc = persist.tile([P, NPASS, 2, 2, H + 1], F32)  # [q, pass, hf, j2, h|s]
    w_all = persist.tile([P, 2 * NPASS, H], F32)  # [q, subtile, h]
    rsq_all = persist.tile([P, 2 * NPASS], F32)
    rr_all = persist.tile([P, 2 * NPASS], F32)
    rr_i = persist.tile([P, 2 * NPASS], I32)
    o_all = persist.tile([P, 2 * NPASS, H], F32)

    xpool = ctx.enter_context(tc.tile_pool(name="xstream", bufs=1))
    epool = ctx.enter_context(tc.tile_pool(name="epool", bufs=6))
    small = ctx.enter_context(tc.tile_pool(name="small", bufs=4))

    # ---- const DMAs; ones column of V_aug.  Weights ship as ONE packed
    # tensor (per-partition rows are 12KB contiguous: no small-descriptor
    # penalty); the c0-1 slice of the four k/q weights goes first ----
    nc.sync.dma_start(out=wpk_sb[:, 0:4, 0:2, :], in_=wpk_d[:, 0:4, 0:2, :])
    nc.vector.memset(v_sb[:, :, H : H + 1], 1.0)

    x_tiles, r_tiles = [], []
    for b in range(NBLK):
        x_tiles.append(
            xpool.tile([P, DCH, BLKW], F8, tag=f"x{b % 3}", bufs=1, name=f"xt{b}")
        )
        r_tiles.append(
            xpool.tile([P, DCH, BLKW], F8, tag=f"r{b % 3}", bufs=1, name=f"rt{b}")
        )
    # block-0 x8/r8 interleaved in c-pair slabs: the residual projections
    # pipeline with the DMA instead of waiting for the whole r8 block
    for cp in range(DCH // 2):
        nc.sync.dma_start(
            out=x_tiles[0][:, 2 * cp : 2 * cp + 2, :],
            in_=x8_d[:, 2 * cp : 2 * cp + 2, 0:BLKW],
        )
        if cp == 0:
            nc.sync.dma_start(
                out=wpk_sb[:, 0:4, 2:DCH, :], in_=wpk_d[:, 0:4, 2:DCH, :]
            )
        if cp == 1:
            nc.sync.dma_start(out=wpk_sb[:, 4:6], in_=wpk_d[:, 4:6])
        if cp == 2:
            nc.sync.dma_start(
                out=rmsw_bc,
                in_=bass.AP(tensor=rmsw.tensor, offset=0, ap=[[0, P], [1, H]]),
            )
        nc.sync.dma_start(
            out=r_tiles[0][:, 2 * cp : 2 * cp + 2, :],
            in_=r8_d[:, 2 * cp : 2 * cp + 2, 0:BLKW],
        )

    def x_slabs(b):
        for h2 in range(2):
            nc.sync.dma_start(
                out=x_tiles[b][:, h2 * 8 : (h2 + 1) * 8, :],
                in_=x8_d[:, h2 * 8 : (h2 + 1) * 8, b * BLKW : (b + 1) * BLKW],
            )
            nc.sync.dma_start(
                out=r_tiles[b][:, h2 * 8 : (h2 + 1) * 8, :],
                in_=r8_d[:, h2 * 8 : (h2 + 1) * 8, b * BLKW : (b + 1) * BLKW],
            )

    x_slabs(1)

    psum = ctx.enter_context(tc.tile_pool(name="psum", space="PSUM", bufs=1))

    def proj_copy_k(accm, accr, b, g):
        # hw: ALU ops may read only ONE input from PSUM -> copy main first,
        # then accumulate the scaled residual in place
        sl = slice(b * BLKW + g * 512, b * BLKW + (g + 1) * 512)
        nc.vector.tensor_copy(kT_sb[:, sl], accm)
        nc.vector.scalar_tensor_tensor(
            kT_sb[:, sl], accr, 1.0 / 32.0, kT_sb[:, sl], op0=OP.mult, op1=OP.add
        )
        # second-half heads to base partition 0 via the DVE shuffle network
        # (no DMA: the serial DMA queue is full of x slabs at this point)
        nc.vector.stream_shuffle(kT2_sb[:, sl], kT_sb[64:128, sl], list(range(32)))

    def kq_mm(acc, w_t, xt, cp, g, st, sp):
        nc.tensor.matmul(
            acc,
            w_t[:, 2 * cp : 2 * cp + 2, :],
            xt[:, 2 * cp : 2 * cp + 2, g * 512 : (g + 1) * 512],
            start=st,
            stop=sp,
            perf_mode=DR,
        )

    # ---- block 0: K and Q projections c-inner in the four (idle) U banks,
    # pipelined with the x8 c-pair DMAs so the PE starts early and ramps ----
    kaccm = [
        psum.tile([P, 512], F32, tag=f"u{g}", bufs=1, name=f"kaccm{g}")
        for g in range(2)
    ]
    qaccm = [
        psum.tile([P, 512], F32, tag=f"u{2 + g}", bufs=1, name=f"qaccm{g}")
        for g in range(2)
    ]
    kaccr = [
        psum.tile([P, 512], F32, tag="s", bufs=2, name=f"kaccr{g}")
        for g in range(2)
    ]
    qaccr = [
        psum.tile([P, 512], F32, tag="work", bufs=2, name=f"qaccr{g}")
        for g in range(2)
    ]
    # per c-pair: main, x8*ws residual, r8*w residual — everything only needs
    # that c-pair of x8/r8, so the whole phase is DMA-paced
    for cp in range(DCH // 2):
        st, sp = cp == 0, cp == DCH // 2 - 1
        for g in range(2):
            kq_mm(kaccm[g], wk_sb, x_tiles[0], cp, g, st, sp)
            kq_mm(qaccm[g], wq_sb, x_tiles[0], cp, g, st, sp)
            kq_mm(kaccr[g], wks_sb, x_tiles[0], cp, g, st, False)
            kq_mm(qaccr[g], wqs_sb, x_tiles[0], cp, g, st, False)
            kq_mm(kaccr[g], wk_sb, r_tiles[0], cp, g, False, sp)
            kq_mm(qaccr[g], wq_sb, r_tiles[0], cp, g, False, sp)
    for g in range(2):
        proj_copy_k(kaccm[g], kaccr[g], 0, g)
        qsl = slice(g * 512, (g + 1) * 512)
        nc.scalar.copy(qT_sb[:, qsl], qaccm[g])
        nc.vector.scalar_tensor_tensor(
            qT_sb[:, qsl], qaccr[g], 1.0 / 32.0, qT_sb[:, qsl],
            op0=OP.mult, op1=OP.add,
        )
        nc.vector.stream_shuffle(
            qT2_sb[:, g * 512 : (g + 1) * 512],
            qT_sb[64:128, g * 512 : (g + 1) * 512],
            list(range(32)),
        )
    x_slabs(2)
    x_slabs(3)

    def kproj_block(b):
        """K projection for block b (fp8 main + residual, two work banks)."""
        for g in range(2):
            accm = psum.tile([P, 512], F32, tag="work", bufs=2, name=f"km{b}{g}")
            accr = psum.tile([P, 512], F32, tag="work", bufs=2, name=f"kr{b}{g}")
            for cp in range(DCH // 2):
                kq_mm(accm, wk_sb, x_tiles[b], cp, g, cp == 0, cp == DCH // 2 - 1)
            for cp in range(DCH // 2):
                kq_mm(accr, wks_sb, x_tiles[b], cp, g, cp == 0, False)
            for cp in range(DCH // 2):
                kq_mm(accr, wk_sb, r_tiles[b], cp, g, False, cp == DCH // 2 - 1)
            proj_copy_k(accm, accr, b, g)

    def vproj_block(b):
        """V projection for block b: fp8 main term plus (1/32)-scaled
        residual terms, two work banks, j-outer (sequential groups/bank)."""
        for g in range(2):
            vmain = psum.tile([P, 4, P], F32, tag="work", bufs=2, name=f"vm{b}{g}")
            vres = psum.tile([P, 4, P], F32, tag="work", bufs=2, name=f"vr{b}{g}")
            for j in range(4):
                ksl = slice(g * 512 + j * P, g * 512 + (j + 1) * P)
                for cp in range(DCH // 2):
                    csl = slice(2 * cp, 2 * cp + 2)
                    nc.tensor.matmul(
                        vmain[:, j, :],
                        x_tiles[b][:, csl, ksl],
                        wv_sb[:, csl, :],
                        start=(cp == 0),
                        stop=(cp == DCH // 2 - 1),
                        perf_mode=DR,
                    )
                for cp in range(DCH // 2):
                    csl = slice(2 * cp, 2 * cp + 2)
                    nc.tensor.matmul(
                        vres[:, j, :],
                        x_tiles[b][:, csl, ksl],
                        ws_sb[:, csl, :],
                        start=(cp == 0),
                        stop=False,
                        perf_mode=DR,
                    )
                for cp in range(DCH // 2):
                    csl = slice(2 * cp, 2 * cp + 2)
                    nc.tensor.matmul(
                        vres[:, j, :],
                        r_tiles[b][:, csl, ksl],
                        wv_sb[:, csl, :],
                        start=False,
                        stop=(cp == DCH // 2 - 1),
                        perf_mode=DR,
                    )
            vsl = v_sb[:, b * 8 + g * 4 : b * 8 + (g + 1) * 4, 0:H]
            nc.vector.tensor_copy(vsl, vmain)
            nc.vector.scalar_tensor_tensor(
                vsl, vres, 1.0 / 32.0, vsl, op0=OP.mult, op1=OP.add
            )

    stage = int(os.environ.get("KSTAGE", "3"))
    if stage <= 1:
        vproj_block(0)
        for b in range(1, NBLK):
            kproj_block(b)
            vproj_block(b)
        o_dbg = small.tile([P, H], F32, tag="odbg")
        nc.vector.tensor_copy(o_dbg, kT_sb[:, 0:H])
        for t in range(2 * NPASS):
            nc.sync.dma_start(out=out_d[t * P : (t + 1) * P, :], in_=o_dbg)
        return

    def attend_block_pass(b, p):
        """Pass p (256 q cols), all 8 key chunks of block b -> U psum, then
        drain-add into u_acc."""
        q0 = p * PW
        u_ps = [
            psum.tile([P, 512], F32, tag=f"u{s}", bufs=1, name=f"u{b}_{p}_{s}")
            for s in range(4)
        ]
        for cl in range(8):
            ch = b * 8 + cl
            s_ps = psum.tile([P, 512], F32, tag="s", bufs=2)
            with tc.high_priority(offset=5000):
                nc.tensor.matmul(
                    s_ps[:, 0:PW],
                    kT_sb[0:H2, ch * P : (ch + 1) * P],
                    qT_sb[0:H2, q0 : q0 + PW],
                )
                nc.tensor.matmul(
                    s_ps[:, PW : 2 * PW],
                    kT2_sb[:, ch * P : (ch + 1) * P],
                    qT2_sb[:, q0 : q0 + PW],
                )
                e_sb = epool.tile([P, 512], BF16, tag="e")
                nc.scalar.activation(e_sb, s_ps, AF.Exp, scale=SCALE_EXP)
            for hf in range(2):
                for j2 in range(2):
                    nc.tensor.matmul(
                        u_ps[hf * 2 + j2][:, 0 : H + 1],
                        e_sb[:, hf * PW + j2 * P : hf * PW + (j2 + 1) * P],
                        v_sb[:, ch, :],
                        start=(cl == 0),
                        stop=(cl == 7),
                    )
        for slot in range(4):
            hf, j2 = divmod(slot, 2)
            dst = u_acc[:, p, hf, j2, :]
            src = u_ps[slot][:, 0 : H + 1]
            if b == 0:
                nc.vector.tensor_copy(dst, src)
            else:
                nc.vector.tensor_tensor(dst, src, dst, op=OP.add)

    def post_pass(p):
        """Normalize/combine for pass p's two q-subtiles (DVE + Pool only)."""
        for j2 in range(2):
            t = p * 2 + j2
            s1 = u_acc[:, p, 0, j2, H : H + 1]
            s2 = u_acc[:, p, 1, j2, H : H + 1]
            r2 = small.tile([P, 1], F32, tag="r2")
            nc.vector.reciprocal(r2, s2)
            al = small.tile([P, 1], F32, tag="al")
            nc.vector.scalar_tensor_tensor(
                al, s1, -lam, r2, op0=OP.mult, op1=OP.mult
            )
            nc.vector.scalar_tensor_tensor(
                w_all[:, t, :],
                u_acc[:, p, 1, j2, 0:H],
                al,
                u_acc[:, p, 0, j2, 0:H],
                op0=OP.mult,
                op1=OP.add,
            )
            sq = small.tile([P, H], F32, tag="sq")
            nc.gpsimd.tensor_tensor(sq, w_all[:, t, :], w_all[:, t, :], op=OP.mult)
            nc.vector.tensor_reduce(
                rsq_all[:, t : t + 1], sq, axis=mybir.AxisListType.X, op=OP.add
            )
        # rr = rsqrt(ssq) on DVE (bit-trick + 2 Newton steps, ~5e-6 rel):
        # keeps the ACT exp stream free of Sqrt table switches
        t0 = p * 2
        ss = rsq_all[:, t0 : t0 + 2]
        yi = rr_i[:, t0 : t0 + 2]
        y = yi.bitcast(F32)
        nc.vector.tensor_scalar(
            yi, ss.bitcast(I32), 1, None, op0=OP.arith_shift_right
        )
        nc.vector.tensor_scalar(yi, yi, -1, 0x5F3759DF, op0=OP.mult, op1=OP.add)
        nt = small.tile([P, 2], F32, tag="nt")
        for _ in range(2):
            nc.vector.tensor_tensor(nt, ss, y, op=OP.mult)
            nc.vector.tensor_tensor(nt, nt, y, op=OP.mult)
            nc.vector.tensor_scalar(nt, nt, -0.5, 1.5, op0=OP.mult, op1=OP.add)
            nc.vector.tensor_tensor(y, y, nt, op=OP.mult)
        nc.vector.tensor_copy(rr_all[:, t0 : t0 + 2], y)
        for t in (t0, t0 + 1):
            nc.vector.scalar_tensor_tensor(
                o_all[:, t, :], w_all[:, t, :], rr_all[:, t : t + 1], rmsw_bc,
                op0=OP.mult, op1=OP.mult,
            )
        nc.scalar.dma_start(
            out=out_d[t0 * P : (t0 + 2) * P, :].rearrange("(t p) h -> p t h", p=P),
            in_=o_all[:, t0 : t0 + 2, :],
        )

    # ---- main schedule: per block: 4 passes, with the next block's K/V
    # projections emitted between passes (they fill PE slack while the pass
    # stream drains through ACT) ----
    vproj_block(0)
    for b in range(NBLK):
        for p in range(NPASS):
            attend_block_pass(b, p)
            if b < NBLK - 1:
                if p == 0:
                    kproj_block(b + 1)
                elif p == 1:
                    vproj_block(b + 1)
            if b == NBLK - 1 and stage >= 3:
                post_pass(p)


def build(lam: float):
    from concourse._compat import axon_active

    nc = bacc.Bacc(
        "TRN2",
        target_bir_lowering=False,
        debug=not axon_active(),
        num_devices=NCORES,
    )
    with tile.TileContext(nc) as tc:
        with ExitStack() as ctx:
            _emit(ctx, tc, lam)
    nc.compile()
    return nc


def make_in_maps(x, Wq, Wk, Wv, rms_weight):
    f8 = ml_dtypes.float8_e4m3
    x = np.asarray(x, dtype=np.float32)
    xT = np.ascontiguousarray(x.transpose(0, 2, 1))  # [B, D, S]
    xP = xT.reshape(B, DCH, P, S).transpose(0, 2, 1, 3)  # [B, p, c, S]

    def wsplit(W):
        # w'[p, c, h] = 64*W[h, c*128+p]; fp8 main + fp8 32x-scaled residual
        WT = np.asarray(W, np.float32).T * WSCALE  # [D, H]
        wp = np.ascontiguousarray(WT.reshape(DCH, P, H).transpose(1, 0, 2))
        w8 = wp.astype(f8)
        ws8 = ((wp - w8.astype(np.float32)) * 32.0).astype(f8)
        return np.ascontiguousarray(w8), np.ascontiguousarray(ws8)

    wq8, wqs8 = wsplit(Wq)
    wk8, wks8 = wsplit(Wk)
    wv8, ws8 = wsplit(Wv)
    wpk = np.ascontiguousarray(
        np.stack([wk8, wq8, wks8, wqs8, wv8, ws8], axis=1)
    )
    c_ = 1.0 - LAMBDA_INIT
    rw = np.ascontiguousarray(
        np.asarray(rms_weight, np.float32) * c_ * math.sqrt(H)
    ).astype(np.float32)

    in_maps = []
    for core in range(NCORES):
        b, qb = divmod(core, NCORES // B)
        # own query block's columns first; key order is attention-irrelevant
        cols = np.r_[qb * QSHARD : (qb + 1) * QSHARD,
                     0 : qb * QSHARD, (qb + 1) * QSHARD : S]
        xc = xP[b][:, :, cols]
        x8 = np.ascontiguousarray(xc.astype(f8))
        r8 = np.ascontiguousarray(
            ((xc - x8.astype(np.float32)) * 32.0).astype(f8)
        )
        in_maps.append(
            {"x8": x8, "r8": r8, "wpk": wpk, "rmsw": rw}
        )
    # e4m3 and e4m3fn are byte-identical over our value range (<224); jax's
    # transfer path only accepts the fn variant, the device reads raw bytes
    fn = ml_dtypes.float8_e4m3fn
    for m in in_maps:
        for k, v in m.items():
            if v.dtype == f8:
                m[k] = v.view(fn)
    return in_maps


def kernel(x, Wq, Wk, Wv, lambda_q1, lambda_q2, lambda_k1, lambda_k2, rms_weight):
    lq1 = np.asarray(lambda_q1, np.float32)
    lq2 = np.asarray(lambda_q2, np.float32)
    lk1 = np.asarray(lambda_k1, np.float32)
    lk2 = np.asarray(lambda_k2, np.float32)
    lam = float(
        np.exp(np.dot(lq1, lk1)) - np.exp(np.dot(lq2, lk2)) + LAMBDA_INIT
    )
    nc = build(lam)
    in_maps = make_in_maps(x, Wq, Wk, Wv, rms_weight)
    res = bass_utils.run_bass_kernel_spmd(nc, in_maps, core_ids=list(range(NCORES)))
    out = np.empty((B, S, H), np.float32)
    for core in range(NCORES):
        b, qb = divmod(core, NCORES // B)
        out[b, qb * QSHARD : (qb + 1) * QSHARD] = res.results[core]["out"]
    return out


# revision 37
# speedup vs baseline: 1.8327x; 1.0209x over previous
"""DiffAttn (differential attention) Trainium2 Bass kernel — v3.

Self-contained: kernel(**inputs) takes the FULL unsharded inputs as numpy
arrays and returns the FULL output [2, 4096, 128] float32.

Sharding: 8 cores = (batch in {0,1}) x (query-block of 1024 rows). Each core
is FULLY independent (no collectives): it streams the whole batch's
activations (host-permuted to [p][c][q] with its own query block's columns
first), projects K and V for all 4096 keys locally, Q for its own 1024
queries, and runs both softmaxes + combined PV + RMSNorm for its queries.
Replicating the K/V projections beats the AllGather under the timeline model
(the collective is priced at ~67us, mostly unoverlappable) and keeps every
core's program trivially SPMD-identical.

fp8 projections (DoubleRow, 4x PE throughput): x and the weights are shipped
as e4m3 (weights pre-scaled by 64 so they sit in fp8's normal range; the 64^2
on the scores is folded into the softmax exp scale, exactly).  Q/K tolerate
plain fp8 (softmax output error ~0.1%).  V needs more precision, so V is
computed as  x8@wv8 + (x8@ws8 + r8@wv8)/32  where r8 = fp8(32*(x - x8)) and
ws8 = fp8(32*(wv' - wv8)) — a first-order residual expansion; the dropped
r*s cross term is ~0.1%.  The global 64x on V cancels in RMSNorm.

Attention layout: scores are computed TRANSPOSED ([sk, q], keys on
partitions) so exp(scores) feeds PV as the STATIONARY operand against a
MOVING V_aug = [V | ones] (129 cols); the ones column makes each PV
accumulation also produce the softmax denominator in column 128 of U — no
separate row-sum matmuls and no transposes in the post phase.  Key order is
attention-irrelevant, so K/V chunks process in host-permuted order.
Second-half heads live in base-partition-0 copies (kT2/qT2, via SBUF
partition-move DMAs): matmuls that share a PSUM bank must share a PE row
base (hardware tile-position rule).

Post: attn = U1/s1 - lam*U2/s2; RMSNorm(attn) == RMSNorm(s1*attn), so
w := U1 - lam*(s1/s2)*U2 needs no 1/s1 division (torch eps is ~1e-5 of
mean(attn^2) here).  out = w * rsqrt(sum_h w^2) * [rmsw*(1-li)*sqrt(H)].
rsqrt runs on DVE (bit-trick + 2 Newton steps, ~5e-6 rel) so the ACT exp
stream never switches activation tables; the post runs on DVE/Pool per pass,
fully overlapped except the last pass's short chain.

PSUM (8 banks): s(2 bufs, 1 bank each) + 4 U slot banks + work(2 bufs) for
the next block's K/V accumulators.  Block 0's K/Q run c-inner in the four
U banks (idle until attention starts) so the PE ramps while x streams in.
"""

import math
import os
import sys
from contextlib import ExitStack

import numpy as np

for _p in ("/root/.axon_site/_ro/trn_rl_repo", "/opt/trn_rl_repo"):
    if os.path.isdir(_p) and _p not in sys.path:
        sys.path.append(_p)

import ml_dtypes  # noqa: E402

import concourse.bass as bass  # noqa: E402
import concourse.mybir as mybir  # noqa: E402
import concourse.tile as tile  # noqa: E402
from concourse import bacc, bass_utils  # noqa: E402

B, S, D, H = 2, 4096, 2048, 128
H2 = H // 2  # 64
P = 128
NCORES = 8
QSHARD = 1024  # q rows per core
DCH = D // P  # 16 d-chunks
NKCH = S // P  # 32 key chunks of 128
NBLK, BLKW = 4, 1024  # key blocks (1024 keys each)
NPASS, PW = 4, 256  # query passes of 256 columns (U psum = 4 banks)
WSCALE = 64.0  # host-side weight prescale (exact power of 2)

LAMBDA_INIT = 0.8 - 0.6 * math.exp(-0.3 * 12)
SCALE = 1.0 / math.sqrt(H2)
SCALE_EXP = SCALE / (WSCALE * WSCALE)  # exp input is (64q).(64k)

F32 = mybir.dt.float32
BF16 = mybir.dt.bfloat16
F8 = mybir.dt.float8e4
I32 = mybir.dt.int32

AF = mybir.ActivationFunctionType
OP = mybir.AluOpType
DR = mybir.MatmulPerfMode.DoubleRow


def _emit(ctx: ExitStack, tc: "tile.TileContext", lam: float):  # noqa: C901
    nc = tc.nc

    x8_d = nc.dram_tensor("x8", (P, DCH, S), F8, kind="ExternalInput").ap()
    r8_d = nc.dram_tensor("r8", (P, DCH, S), F8, kind="ExternalInput").ap()
    wpk_d = nc.dram_tensor("wpk", (P, 6, DCH, H), F8, kind="ExternalInput").ap()
    rmsw = nc.dram_tensor("rmsw", (H,), F32, kind="ExternalInput").ap()
    out_d = nc.dram_tensor("out", (QSHARD, H), F32, kind="ExternalOutput").ap()

    # ---- persistent SBUF ----
    consts = ctx.enter_context(tc.tile_pool(name="consts", bufs=1))
    persist = ctx.enter_context(tc.tile_pool(name="persist", bufs=1))

    wpk_sb = consts.tile([P, 6, DCH, H], F8)
    wk_sb, wq_sb, wks_sb, wqs_sb, wv_sb, ws_sb = (
        wpk_sb[:, i] for i in range(6)
    )
    rmsw_bc = consts.tile([P, H], F32)

    kT_sb = persist.tile([P, S], BF16)  # [h, key]
    qT_sb = persist.tile([P, QSHARD], BF16)  # [h, q]
    # second-half heads relocated to base partition 0 (PE tile-position rule)
    kT2_sb = persist.tile([64, S], BF16)
    qT2_sb = persist.tile([64, QSHARD], BF16)
    v_sb = persist.tile([P, NKCH, H + 1], BF16)  # [sk%128, chunk, h|1]
    u_acc = persist.tile([P, NPASS, 2, 2, H + 1], F32)  # [q, pass, hf, j2, h|s]
    w_all = persist.tile([P, 2 * NPASS, H], F32)  # [q, subtile, h]
    rsq_all = persist.tile([P, 2 * NPASS], F32)
    rr_all = persist.tile([P, 2 * NPASS], F32)
    rr_i = persist.tile([P, 2 * NPASS], I32)
    o_all = persist.tile([P, 2 * NPASS, H], F32)

    xpool = ctx.enter_context(tc.tile_pool(name="xstream", bufs=1))
    epool = ctx.enter_context(tc.tile_pool(name="epool", bufs=10))
    small = ctx.enter_context(tc.tile_pool(name="small", bufs=4))

    # ---- const DMAs; ones column of V_aug.  Weights ship as ONE packed
    # tensor (per-partition rows are 12KB contiguous: no small-descriptor
    # penalty); the c0-1 slice of the four k/q weights goes first ----
    nc.sync.dma_start(out=wpk_sb[:, 0:4, 0:2, :], in_=wpk_d[:, 0:4, 0:2, :])
    nc.vector.memset(v_sb[:, :, H : H + 1], 1.0)

    x_tiles, r_tiles = [], []
    for b in range(NBLK):
        x_tiles.append(
            xpool.tile([P, DCH, BLKW], F8, tag=f"x{b % 3}", bufs=1, name=f"xt{b}")
        )
        r_tiles.append(
            xpool.tile([P, DCH, BLKW], F8, tag=f"r{b % 3}", bufs=1, name=f"rt{b}")
        )
    # block-0 x8/r8 interleaved in c-pair slabs: the residual projections
    # pipeline with the DMA instead of waiting for the whole r8 block
    for cp in range(DCH // 2):
        nc.sync.dma_start(
            out=x_tiles[0][:, 2 * cp : 2 * cp + 2, :],
            in_=x8_d[:, 2 * cp : 2 * cp + 2, 0:BLKW],
        )
        if cp == 0:
            nc.sync.dma_start(
                out=wpk_sb[:, 0:4, 2:DCH, :], in_=wpk_d[:, 0:4, 2:DCH, :]
            )
        if cp == 1:
            nc.sync.dma_start(out=wpk_sb[:, 4:6], in_=wpk_d[:, 4:6])
        if cp == 2:
            nc.sync.dma_start(
                out=rmsw_bc,
                in_=bass.AP(tensor=rmsw.tensor, offset=0, ap=[[0, P], [1, H]]),
            )
        nc.sync.dma_start(
            out=r_tiles[0][:, 2 * cp : 2 * cp + 2, :],
            in_=r8_d[:, 2 * cp : 2 * cp + 2, 0:BLKW],
        )

    def x_slabs(b):
        for h2 in range(2):
            nc.sync.dma_start(
                out=x_tiles[b][:, h2 * 8 : (h2 + 1) * 8, :],
                in_=x8_d[:, h2 * 8 : (h2 + 1) * 8, b * BLKW : (b + 1) * BLKW],
            )
            nc.sync.dma_start(
                out=r_tiles[b][:, h2 * 8 : (h2 + 1) * 8, :],
                in_=r8_d[:, h2 * 8 : (h2 + 1) * 8, b * BLKW : (b + 1) * BLKW],
            )

    x_slabs(1)

    psum = ctx.enter_context(tc.tile_pool(name="psum", space="PSUM", bufs=1))

    def proj_copy_k(accm, accr, b, g):
        # hw: ALU ops may read only ONE input from PSUM -> copy main first,
        # then accumulate the scaled residual in place
        sl = slice(b * BLKW + g * 512, b * BLKW + (g + 1) * 512)
        nc.vector.tensor_copy(kT_sb[:, sl], accm)
        nc.vector.scalar_tensor_tensor(
            kT_sb[:, sl], accr, 1.0 / 32.0, kT_sb[:, sl], op0=OP.mult, op1=OP.add
        )
        # second-half heads to base partition 0 via the DVE shuffle network
        # (no DMA: the serial DMA queue is full of x slabs at this point)
        nc.vector.stream_shuffle(kT2_sb[:, sl], kT_sb[64:128, sl], list(range(32)))

    def kq_mm(acc, w_t, xt, cp, g, st, sp):
        nc.tensor.matmul(
            acc,
            w_t[:, 2 * cp : 2 * cp + 2, :],
            xt[:, 2 * cp : 2 * cp + 2, g * 512 : (g + 1) * 512],
            start=st,
            stop=sp,
            perf_mode=DR,
        )

    # ---- block 0: K and Q projections c-inner in the four (idle) U banks,
    # pipelined with the x8 c-pair DMAs so the PE starts early and ramps ----
    kaccm = [
        psum.tile([P, 512], F32, tag=f"u{g}", bufs=1, name=f"kaccm{g}")
        for g in range(2)
    ]
    qaccm = [
        psum.tile([P, 512], F32, tag=f"u{2 + g}", bufs=1, name=f"qaccm{g}")
        for g in range(2)
    ]
    kaccr = [
        psum.tile([P, 512], F32, tag="s", bufs=2, name=f"kaccr{g}")
        for g in range(2)
    ]
    qaccr = [
        psum.tile([P, 512], F32, tag="work", bufs=2, name=f"qaccr{g}")
        for g in range(2)
    ]
    # per c-pair: main, x8*ws residual, r8*w residual — everything only needs
    # that c-pair of x8/r8, so the whole phase is DMA-paced
    for cp in range(DCH // 2):
        st, sp = cp == 0, cp == DCH // 2 - 1
        for g in range(2):
            kq_mm(kaccm[g], wk_sb, x_tiles[0], cp, g, st, sp)
            kq_mm(qaccm[g], wq_sb, x_tiles[0], cp, g, st, sp)
            kq_mm(kaccr[g], wks_sb, x_tiles[0], cp, g, st, False)
            kq_mm(qaccr[g], wqs_sb, x_tiles[0], cp, g, st, False)
            kq_mm(kaccr[g], wk_sb, r_tiles[0], cp, g, False, sp)
            kq_mm(qaccr[g], wq_sb, r_tiles[0], cp, g, False, sp)
    for g in range(2):
        proj_copy_k(kaccm[g], kaccr[g], 0, g)
        qsl = slice(g * 512, (g + 1) * 512)
        nc.scalar.copy(qT_sb[:, qsl], qaccm[g])
        nc.vector.scalar_tensor_tensor(
            qT_sb[:, qsl], qaccr[g], 1.0 / 32.0, qT_sb[:, qsl],
            op0=OP.mult, op1=OP.add,
        )
        nc.vector.stream_shuffle(
            qT2_sb[:, g * 512 : (g + 1) * 512],
            qT_sb[64:128, g * 512 : (g + 1) * 512],
            list(range(32)),
        )
    x_slabs(2)
    x_slabs(3)

    def kproj_block(b):
        """K projection for block b (fp8 main + residual, two work banks)."""
        for g in range(2):
            accm = psum.tile([P, 512], F32, tag="work", bufs=2, name=f"km{b}{g}")
            accr = psum.tile([P, 512], F32, tag="work", bufs=2, name=f"kr{b}{g}")
            for cp in range(DCH // 2):
                kq_mm(accm, wk_sb, x_tiles[b], cp, g, cp == 0, cp == DCH // 2 - 1)
            for cp in range(DCH // 2):
                kq_mm(accr, wks_sb, x_tiles[b], cp, g, cp == 0, False)
            for cp in range(DCH // 2):
                kq_mm(accr, wk_sb, r_tiles[b], cp, g, False, cp == DCH // 2 - 1)
            proj_copy_k(accm, accr, b, g)

    def vproj_block(b):
        """V projection for block b: fp8 main term plus (1/32)-scaled
        residual terms, two work banks, j-outer (sequential groups/bank)."""
        for g in range(2):
            vmain = psum.tile([P, 4, P], F32, tag="work", bufs=2, name=f"vm{b}{g}")
            vres = psum.tile([P, 4, P], F32, tag="work", bufs=2, name=f"vr{b}{g}")
            for j in range(4):
                ksl = slice(g * 512 + j * P, g * 512 + (j + 1) * P)
                for cp in range(DCH // 2):
                    csl = slice(2 * cp, 2 * cp + 2)
                    nc.tensor.matmul(
                        vmain[:, j, :],
                        x_tiles[b][:, csl, ksl],
                        wv_sb[:, csl, :],
                        start=(cp == 0),
                        stop=(cp == DCH // 2 - 1),
                        perf_mode=DR,
                    )
                for cp in range(DCH // 2):
                    csl = slice(2 * cp, 2 * cp + 2)
                    nc.tensor.matmul(
                        vres[:, j, :],
                        x_tiles[b][:, csl, ksl],
                        ws_sb[:, csl, :],
                        start=(cp == 0),
                        stop=False,
                        perf_mode=DR,
                    )
                for cp in range(DCH // 2):
                    csl = slice(2 * cp, 2 * cp + 2)
                    nc.tensor.matmul(
                        vres[:, j, :],
                        r_tiles[b][:, csl, ksl],
                        wv_sb[:, csl, :],
                        start=False,
                        stop=(cp == DCH // 2 - 1),
                        perf_mode=DR,
                    )
            vsl = v_sb[:, b * 8 + g * 4 : b * 8 + (g + 1) * 4, 0:H]
            nc.vector.tensor_copy(vsl, vmain)
            nc.vector.scalar_tensor_tensor(
                vsl, vres, 1.0 / 32.0, vsl, op0=OP.mult, op1=OP.add
            )

    stage = int(os.environ.get("KSTAGE", "3"))
    if stage <= 1:
        vproj_block(0)
        for b in range(1, NBLK):
            kproj_block(b)
            vproj_block(b)
        o_dbg = small.tile([P, H], F32, tag="odbg")
        nc.vector.tensor_copy(o_dbg, kT_sb[:, 0:H])
        for t in range(2 * NPASS):
            nc.sync.dma_start(out=out_d[t * P : (t + 1) * P, :], in_=o_dbg)
        return

    def attend_block_pass(b, p):
        """Pass p (256 q cols), all 8 key chunks of block b -> U psum, then
        drain-add into u_acc."""
        q0 = p * PW
        u_ps = [
            psum.tile([P, 512], F32, tag=f"u{s}", bufs=1, name=f"u{b}_{p}_{s}")
            for s in range(4)
        ]
        for cl in range(8):
            ch = b * 8 + cl
            s_ps = psum.tile([P, 512], F32, tag="s", bufs=2)
            with tc.high_priority(offset=5000):
                nc.tensor.matmul(
                    s_ps[:, 0:PW],
                    kT_sb[0:H2, ch * P : (ch + 1) * P],
                    qT_sb[0:H2, q0 : q0 + PW],
                )
                nc.tensor.matmul(
                    s_ps[:, PW : 2 * PW],
                    kT2_sb[:, ch * P : (ch + 1) * P],
                    qT2_sb[:, q0 : q0 + PW],
                )
                e_sb = epool.tile([P, 512], BF16, tag="e")
                nc.scalar.activation(e_sb, s_ps, AF.Exp, scale=SCALE_EXP)
            for hf in range(2):
                for j2 in range(2):
                    nc.tensor.matmul(
                        u_ps[hf * 2 + j2][:, 0 : H + 1],
                        e_sb[:, hf * PW + j2 * P : hf * PW + (j2 + 1) * P],
                        v_sb[:, ch, :],
                        start=(cl == 0),
                        stop=(cl == 7),
                    )
        with tc.high_priority(offset=600):
            for slot in range(4):
                hf, j2 = divmod(slot, 2)
                dst = u_acc[:, p, hf, j2, :]
                src = u_ps[slot][:, 0 : H + 1]
                if b == 0:
                    nc.vector.tensor_copy(dst, src)
                else:
                    nc.vector.tensor_tensor(dst, src, dst, op=OP.add)

    def post_pass(p):
        """Normalize/combine for pass p's two q-subtiles (DVE + Pool only)."""
        for j2 in range(2):
            t = p * 2 + j2
            s1 = u_acc[:, p, 0, j2, H : H + 1]
            s2 = u_acc[:, p, 1, j2, H : H + 1]
            r2 = small.tile([P, 1], F32, tag="r2")
            nc.vector.reciprocal(r2, s2)
            al = small.tile([P, 1], F32, tag="al")
            nc.vector.scalar_tensor_tensor(
                al, s1, -lam, r2, op0=OP.mult, op1=OP.mult
            )
            nc.vector.scalar_tensor_tensor(
                w_all[:, t, :],
                u_acc[:, p, 1, j2, 0:H],
                al,
                u_acc[:, p, 0, j2, 0:H],
                op0=OP.mult,
                op1=OP.add,
            )
            sq = small.tile([P, H], F32, tag="sq")
            nc.gpsimd.tensor_tensor(sq, w_all[:, t, :], w_all[:, t, :], op=OP.mult)
            nc.vector.tensor_reduce(
                rsq_all[:, t : t + 1], sq, axis=mybir.AxisListType.X, op=OP.add
            )
        # rr = rsqrt(ssq) on DVE (bit-trick + 2 Newton steps, ~5e-6 rel):
        # keeps the ACT exp stream free of Sqrt table switches
        t0 = p * 2
        ss = rsq_all[:, t0 : t0 + 2]
        yi = rr_i[:, t0 : t0 + 2]
        y = yi.bitcast(F32)
        nc.vector.tensor_scalar(
            yi, ss.bitcast(I32), 1, None, op0=OP.arith_shift_right
        )
        nc.vector.tensor_scalar(yi, yi, -1, 0x5F3759DF, op0=OP.mult, op1=OP.add)
        nt = small.tile([P, 2], F32, tag="nt")
        for _ in range(2):
            nc.vector.tensor_tensor(nt, ss, y, op=OP.mult)
            nc.vector.tensor_tensor(nt, nt, y, op=OP.mult)
            nc.vector.tensor_scalar(nt, nt, -0.5, 1.5, op0=OP.mult, op1=OP.add)
            nc.vector.tensor_tensor(y, y, nt, op=OP.mult)
        nc.vector.tensor_copy(rr_all[:, t0 : t0 + 2], y)
        for t in (t0, t0 + 1):
            nc.vector.scalar_tensor_tensor(
                o_all[:, t, :], w_all[:, t, :], rr_all[:, t : t + 1], rmsw_bc,
                op0=OP.mult, op1=OP.mult,
            )
        nc.scalar.dma_start(
            out=out_d[t0 * P : (t0 + 2) * P, :].rearrange("(t p) h -> p t h", p=P),
            in_=o_all[:, t0 : t0 + 2, :],
        )

    # ---- main schedule: per block: 4 passes, with the next block's K/V
    # projections emitted between passes (they fill PE slack while the pass
    # stream drains through ACT) ----
    vproj_block(0)
    for b in range(NBLK):
        for p in range(NPASS):
            attend_block_pass(b, p)
            if b < NBLK - 1:
                if p == 1:
                    kproj_block(b + 1)
                elif p == 2:
                    vproj_block(b + 1)
            if b == NBLK - 1 and stage >= 3:
                post_pass(p)


def build(lam: float):
    from concourse._compat import axon_active

    nc = bacc.Bacc(
        "TRN2",
        target_bir_lowering=False,
        debug=not axon_active(),
        num_devices=NCORES,
    )
    with tile.TileContext(nc) as tc:
        with ExitStack() as ctx:
            _emit(ctx, tc, lam)
    nc.compile()
    return nc


def make_in_maps(x, Wq, Wk, Wv, rms_weight):
    f8 = ml_dtypes.float8_e4m3
    x = np.asarray(x, dtype=np.float32)
    xT = np.ascontiguousarray(x.transpose(0, 2, 1))  # [B, D, S]
    xP = xT.reshape(B, DCH, P, S).transpose(0, 2, 1, 3)  # [B, p, c, S]

    def wsplit(W):
        # w'[p, c, h] = 64*W[h, c*128+p]; fp8 main + fp8 32x-scaled residual
        WT = np.asarray(W, np.float32).T * WSCALE  # [D, H]
        wp = np.ascontiguousarray(WT.reshape(DCH, P, H).transpose(1, 0, 2))
        w8 = wp.astype(f8)
        ws8 = ((wp - w8.astype(np.float32)) * 32.0).astype(f8)
        return np.ascontiguousarray(w8), np.ascontiguousarray(ws8)

    wq8, wqs8 = wsplit(Wq)
    wk8, wks8 = wsplit(Wk)
    wv8, ws8 = wsplit(Wv)
    wpk = np.ascontiguousarray(
        np.stack([wk8, wq8, wks8, wqs8, wv8, ws8], axis=1)
    )
    c_ = 1.0 - LAMBDA_INIT
    rw = np.ascontiguousarray(
        np.asarray(rms_weight, np.float32) * c_ * math.sqrt(H)
    ).astype(np.float32)

    in_maps = []
    for core in range(NCORES):
        b, qb = divmod(core, NCORES // B)
        # own query block's columns first; key order is attention-irrelevant
        cols = np.r_[qb * QSHARD : (qb + 1) * QSHARD,
                     0 : qb * QSHARD, (qb + 1) * QSHARD : S]
        xc = xP[b][:, :, cols]
        x8 = np.ascontiguousarray(xc.astype(f8))
        r8 = np.ascontiguousarray(
            ((xc - x8.astype(np.float32)) * 32.0).astype(f8)
        )
        in_maps.append(
            {"x8": x8, "r8": r8, "wpk": wpk, "rmsw": rw}
        )
    # e4m3 and e4m3fn are byte-identical over our value range (<224); jax's
    # transfer path only accepts the fn variant, the device reads raw bytes
    fn = ml_dtypes.float8_e4m3fn
    for m in in_maps:
        for k, v in m.items():
            if v.dtype == f8:
                m[k] = v.view(fn)
    return in_maps


def kernel(x, Wq, Wk, Wv, lambda_q1, lambda_q2, lambda_k1, lambda_k2, rms_weight):
    lq1 = np.asarray(lambda_q1, np.float32)
    lq2 = np.asarray(lambda_q2, np.float32)
    lk1 = np.asarray(lambda_k1, np.float32)
    lk2 = np.asarray(lambda_k2, np.float32)
    lam = float(
        np.exp(np.dot(lq1, lk1)) - np.exp(np.dot(lq2, lk2)) + LAMBDA_INIT
    )
    nc = build(lam)
    in_maps = make_in_maps(x, Wq, Wk, Wv, rms_weight)
    res = bass_utils.run_bass_kernel_spmd(nc, in_maps, core_ids=list(range(NCORES)))
    out = np.empty((B, S, H), np.float32)
    for core in range(NCORES):
        b, qb = divmod(core, NCORES // B)
        out[b, qb * QSHARD : (qb + 1) * QSHARD] = res.results[core]["out"]
    return out
